# revision 12
# baseline (speedup 1.0000x reference)
"""Trainium2 Bass kernel for nn_CRNLayer (CRN-weighted NetVLAD pooling).

Contract: kernel(**inputs) takes the FULL unsharded fp32 inputs and returns the
FULL (64, 32768) fp32 output. Internally: data-parallel over batch N=64 across
8 NeuronCores (8 samples/core), params replicated.

v2 design (per core), all matmuls bf16 with fp32 PSUM accumulation:
  - samples processed in PAIRS (2 per conv matmul, N=512 moving) so each conv
    tap is one 128-contraction x 32-outcol matmul; taps are assigned to the 4
    PE column strips round-robin so up to 4 matmuls stream concurrently
    (measured ~55-61ns/MM vs 217ns serial). Two PSUM banks per pair hold the
    per-strip partial accumulations; DVE combines them, ScalarE does bias+ReLU.
  - 1x1 'wa' conv emits the CRN map directly in q-partition layout (lhsT =
    relu map chunk), so the bilinear 2x upsample matmuls need no scatter DMA.
  - logits / softmax / per-pixel-norm pipeline as v1 (xcm stationary, N=64
    matmuls run at the ~32ns issue floor).
  - VLAD per pair in one PSUM bank: sample A at out-partitions 0:64
    (tile_position (0,0)), sample B at 64:128 ((0,64)) -> 2-way overlap.
  - software-pipelined emit order: conv(p) | tailA(p-1): logits+wa+upsample
    | tailB(p-2): VLAD; Scalar/DVE chains for pair q run under conv(q+1), so
    the PE never starves and HAM stays at K=8/8.
"""
import sys
from contextlib import ExitStack

import numpy as np
import ml_dtypes

try:
    import concourse.bass as bass  # noqa: F401
except ImportError:
    sys.path.insert(0, "/opt/trn_rl_repo")

import concourse.bass as bass
import concourse.mybir as mybir
import concourse.tile as tile
from concourse import bacc
from concourse.bass_utils import run_bass_kernel_spmd

BF16 = ml_dtypes.bfloat16
F32 = np.float32

N, C, W, H, K = 64, 512, 32, 32, 64
P = W * H            # 1024 pixels
Q = 256              # pooled pixels (16x16) per sample
NCORES = 8
SPC = N // NCORES    # samples per core
NPAIR = SPC // 2     # sample pairs per core
EPS = 1e-12

# ---------------------------------------------------------------------------
# conv slot schedule: each slot is one (conv, tap) -> (bank, strip) matmul of
# 128 contraction channels x 32 out-cols. Strips are balanced (21,20,21,21)
# and the emit order round-robins strips for PE col-strip concurrency.
# conv ids: 3 = 7x7 (20 oc), 2 = 5x5 (32 oc), 1 = 3x3 (32 oc)
# ---------------------------------------------------------------------------
_T7 = [(ty, tx) for ty in range(-3, 4) for tx in range(-3, 4)]
_T5 = [(ty, tx) for ty in range(-2, 3) for tx in range(-2, 3)]
_T3 = [(ty, tx) for ty in range(-1, 2) for tx in range(-1, 2)]


def _build_slots():
    # per-strip queues of (conv, ty, tx, bank)
    q = [[], [], [], []]
    q[0] += [(3, ty, tx, 0) for (ty, tx) in _T7[0:8]]
    q[1] += [(3, ty, tx, 0) for (ty, tx) in _T7[8:16]]
    q[2] += [(3, ty, tx, 0) for (ty, tx) in _T7[16:28]]
    q[3] += [(3, ty, tx, 0) for (ty, tx) in _T7[28:49]]
    q[0] += [(2, ty, tx, 1) for (ty, tx) in _T5[0:13]]
    q[1] += [(2, ty, tx, 1) for (ty, tx) in _T5[13:25]]
    q[2] += [(1, ty, tx, 1) for (ty, tx) in _T3]
    # round-robin pop -> emit order rotates strips
    order = []
    idx = [0, 0, 0, 0]
    st = 0
    remaining = sum(len(x) for x in q)
    while remaining:
        tried = 0
        while idx[st] >= len(q[st]) and tried < 4:
            st = (st + 1) % 4
            tried += 1
        conv, ty, tx, bank = q[st][idx[st]]
        order.append((conv, ty, tx, bank, st))
        idx[st] += 1
        remaining -= 1
        st = (st + 1) % 4
    return order


SLOTS = _build_slots()          # 83 entries
NSLOT = len(SLOTS)


def upsample_matrix_16_to_32():
    """1D bilinear (align_corners=False) 16->32 resize matrix, jax semantics."""
    R = np.zeros((32, 16), np.float64)
    for i in range(32):
        pos = (i + 0.5) / 2.0 - 0.5
        lo = int(np.floor(pos))
        f = pos - lo
        tot = (1.0 - f) * (0 <= lo < 16) + f * (0 <= lo + 1 < 16)
        if 0 <= lo < 16:
            R[i, lo] = (1.0 - f) / tot
        if 0 <= lo + 1 < 16:
            R[i, lo + 1] = f / tot
    return R


# --------------------------------------------------------------------------
# device program
# --------------------------------------------------------------------------

def build_nc():
    dt = mybir.dt
    nc = bacc.Bacc("TRN2", target_bir_lowering=False, debug=False)

    d = {}
    d["xcm"] = nc.dram_tensor("xcm", [SPC, C, P], dt.bfloat16, kind="ExternalInput")
    d["xpm"] = nc.dram_tensor("xpm", [SPC, P, C + 1], dt.bfloat16,
                              kind="ExternalInput")
    d["sall"] = nc.dram_tensor("sall", [SPC, P], dt.float32, kind="ExternalInput")
    d["mpad"] = nc.dram_tensor("mpad", [SPC, C, 22 * 22], dt.bfloat16,
                               kind="ExternalInput")
    d["wtv2"] = nc.dram_tensor("wtv2", [128, 4, NSLOT, 32], dt.bfloat16,
                               kind="ExternalInput")
    d["wvt"] = nc.dram_tensor("wvt", [128, 4, K], dt.bfloat16, kind="ExternalInput")
    d["bias96"] = nc.dram_tensor("bias96", [96, 1], dt.float32, kind="ExternalInput")
    d["war"] = nc.dram_tensor("war", [96, 1], dt.bfloat16, kind="ExternalInput")
    d["bup"] = nc.dram_tensor("bup", [128, 2, P], dt.bfloat16, kind="ExternalInput")
    d["cent2"] = nc.dram_tensor("cent2", [128, C], dt.float32, kind="ExternalInput")
    d["ba"] = nc.dram_tensor("ba", [1, 1], dt.float32, kind="ExternalInput")
    d["mask2"] = nc.dram_tensor("mask2", [128, 2], dt.float32, kind="ExternalInput")
    d["mask2T"] = nc.dram_tensor("mask2T", [2, 128], dt.float32, kind="ExternalInput")
    d["out"] = nc.dram_tensor("out", [SPC, K, C], dt.float32, kind="ExternalOutput")

    with tile.TileContext(nc) as tc:
        _emit(nc, tc, d)
    nc.compile()
    return nc


def _emit(nc, tc, d):
    dt = mybir.dt
    fp = dt.float32
    bf = dt.bfloat16
    AF = mybir.ActivationFunctionType
    OP = mybir.AluOpType
    PSUM = bass.MemorySpace.PSUM

    ctx = ExitStack()
    consts = ctx.enter_context(tc.tile_pool(name="consts", bufs=1))
    persist = ctx.enter_context(tc.tile_pool(name="persist", bufs=1))
    work = ctx.enter_context(tc.tile_pool(name="work", bufs=2))
    small = ctx.enter_context(tc.tile_pool(name="small", bufs=4))
    ps = ctx.enter_context(tc.tile_pool(name="ps", bufs=2, space=PSUM))

    # ---- early load: pair 0's channel-major x ----
    xcm_t = [None] * NPAIR
    xcm_t[0] = work.tile([128, 4, 2, P], bf, tag="xcm", bufs=3, name="xcm0")
    for _si in range(2):
        nc.sync.dma_start(
            out=xcm_t[0][:, :, _si, :],
            in_=d["xcm"][_si].rearrange("(cc cp) q -> cp cc q", cp=128))

    # ---- constants ----
    wtv2 = consts.tile([128, 4, NSLOT, 32], bf)
    for cc in range(4):
        nc.sync.dma_start(out=wtv2[:, cc], in_=d["wtv2"][:, cc])
    wvt = consts.tile([128, 4, K], bf)
    nc.sync.dma_start(out=wvt, in_=d["wvt"][:])
    bias96 = consts.tile([96, 1], fp)
    nc.sync.dma_start(out=bias96, in_=d["bias96"][:])
    war = consts.tile([96, 1], bf)
    nc.sync.dma_start(out=war, in_=d["war"][:])
    bup = consts.tile([128, 2, P], bf)
    nc.sync.dma_start(out=bup, in_=d["bup"][:])
    cent2 = consts.tile([128, C], fp)
    nc.sync.dma_start(out=cent2, in_=d["cent2"][:])
    mask2 = consts.tile([128, 2], fp)
    nc.sync.dma_start(out=mask2, in_=d["mask2"][:])
    mask2T = consts.tile([2, 128], fp)
    nc.sync.dma_start(out=mask2T, in_=d["mask2T"][:])
    ba_bc = consts.tile([128, 1], fp)
    ba_ap = d["ba"][:]
    nc.sync.dma_start(
        out=ba_bc,
        in_=bass.AP(tensor=ba_ap.tensor, offset=ba_ap.offset,
                    ap=[[0, 128], ba_ap.ap[1]]))
    zeros = consts.tile([128, 1], fp)
    nc.vector.memset(zeros, 0.0)

    # ---- PE warm-up: junk matmuls so HAM unthrottles before the first convs
    wj = consts.tile([128, 128], bf)
    nc.vector.memset(wj, 0.0)
    xj = consts.tile([128, 512], bf)
    nc.vector.memset(xj, 0.0)
    for i in range(18):
        wps = ps.tile([128, 512], fp, tag="cb0", bufs=2, name=f"warm{i}")
        nc.tensor.matmul(wps, wj, xj, start=True, stop=True)

    # ---- persistent state ----
    xpm = persist.tile([128, SPC, 8, C + 1], bf)     # pixel-major x + ||x|| col
    s_all = persist.tile([128, SPC, 8], fp)          # host 1/max(||x||,eps)
    vlad_all = persist.tile([128, NPAIR, C], fp)     # (s%2)*64+k rows
    q2_all = persist.tile([128, NPAIR], fp)
    nkk_all = persist.tile([128, NPAIR], fp)
    rrk_all = persist.tile([128, NPAIR], fp)
    ttk_all = persist.tile([128, NPAIR], fp)

    # per-pair tiles kept across pipeline stages (2-3 pairs in flight)
    relu84_t = [None] * NPAIR
    mmq_sb = [None] * NPAIR
    a_s = [None] * NPAIR

    # =======================  pipeline stage bodies  =======================

    def emit_loads(p):
        if p >= NPAIR:
            return
        if xcm_t[p] is None:
            xcm_t[p] = work.tile([128, 4, 2, P], bf, tag="xcm", bufs=3,
                                 name=f"xcm{p}")
            for si in range(2):
                nc.sync.dma_start(
                    out=xcm_t[p][:, :, si, :],
                    in_=d["xcm"][2 * p + si].rearrange(
                        "(cc cp) q -> cp cc q", cp=128))
        for s in (2 * p, 2 * p + 1):
            nc.sync.dma_start(
                out=xpm[:, s, :, :],
                in_=d["xpm"][s].rearrange("(pc pp) c -> pp pc c", pp=128))
            nc.sync.dma_start(
                out=s_all[:, s, :],
                in_=d["sall"][s].rearrange("(pc pp) -> pp pc", pp=128))

    mpad_t = [None] * NPAIR

    def emit_pool(p):
        """Host-pooled zero-padded map arrives by DMA."""
        if p >= NPAIR:
            return
        mpad = work.tile([128, 4, 2, 484], bf, tag="mpad", name=f"mpad{p}")
        mpad_t[p] = mpad
        for si in range(2):
            nc.sync.dma_start(
                out=mpad[:, :, si, :],
                in_=d["mpad"][2 * p + si].rearrange("(cc cp) yx -> cp cc yx",
                                                    cp=128))

    def emit_conv(p):
        """332 MMs: 4 cc x 83 slots, strips rotate, 2 PSUM banks."""
        bank0 = ps.tile([128, 512], fp, tag="cb0", name=f"cb0_{p}")
        bank1 = ps.tile([128, 512], fp, tag="cb1", name=f"cb1_{p}")
        banks = (bank0, bank1)
        mpad = mpad_t[p]
        first = {}
        last = {}
        for cc in range(4):
            for sl, (conv, ty, tx, bank, st) in enumerate(SLOTS):
                key = (bank, st)
                if key not in first:
                    first[key] = (cc, sl)
                last[key] = (cc, sl)
        for cc in range(4):
            for sl, (conv, ty, tx, bank, st) in enumerate(SLOTS):
                key = (bank, st)
                mp4 = mpad.rearrange("p cc s (y x) -> p cc s y x", y=22)
                win = mp4[:, cc, :, 3 + ty:19 + ty, 3 + tx:19 + tx]
                nc.tensor.matmul(
                    banks[bank][32 * st:32 * st + 32, :],
                    wtv2[:, cc, sl, :],
                    win,
                    start=(first[key] == (cc, sl)),
                    stop=(last[key] == (cc, sl)),
                    skip_group_check=True,
                    tile_position=(0, 32 * st))
        return banks

    def emit_combine(p, banks):
        """strip-combine + bias + relu -> relu84_t[p] (84, 512) bf16."""
        bank0, bank1 = banks
        scr96 = work.tile([96, 512], fp, tag="scr96")
        # o2 = bank1 strips 0+1 -> rows 0:32
        nc.scalar.copy(scr96[0:32, :], bank1[0:32, :])
        nc.vector.tensor_tensor(scr96[0:32, :], scr96[0:32, :],
                                bank1[32:64, :], op=OP.add)
        # o1 = bank1 strip 2 -> rows 32:64
        nc.scalar.copy(scr96[32:64, :], bank1[64:96, :])
        # o3 (+12 zero pad rows) = sum of 4 full strips of bank0 -> rows 64:96
        nc.scalar.copy(scr96[64:96, :], bank0[0:32, :])
        nc.vector.tensor_tensor(scr96[64:96, :], scr96[64:96, :],
                                bank0[32:64, :], op=OP.add)
        nc.vector.tensor_tensor(scr96[64:96, :], scr96[64:96, :],
                                bank0[64:96, :], op=OP.add)
        nc.vector.tensor_tensor(scr96[64:96, :], scr96[64:96, :],
                                bank0[96:128, :], op=OP.add)
        r = work.tile([96, 512], bf, tag="relu96")
        nc.scalar.activation(r, scr96, AF.Relu, bias=bias96)
        relu84_t[p] = r

    def emit_tailA(p):
        """logits, per-pixel norms, wa + upsample, softmax -> a_s[p]."""
        # --- wa 1x1 conv: mm_q in q-partition layout ---
        mmq_ps = ps.tile([128, 4], fp, tag="sm", bufs=3, name=f"mmq_{p}")
        for qc in range(4):
            nc.tensor.matmul(mmq_ps[:, qc:qc + 1],
                             relu84_t[p][:, 128 * qc:128 * (qc + 1)], war,
                             start=True, stop=True, skip_group_check=True)
        mq = small.tile([128, 4], bf, tag="mmq_sb")
        nc.scalar.copy(mq, mmq_ps)
        mmq_sb[p] = mq

        # --- logits: (x chunk)^T @ wvT -> (128p, 64) per (s, pc);
        # PSUM drains straight through Exp with the 1/||x|| temperature ---
        e_all = work.tile([128, 2, 8, K], bf, tag="e_all")
        for si in range(2):
            s = 2 * p + si
            for pc in range(8):
                saps = ps.tile([128, K], fp, tag="sm", bufs=3)
                for cc in range(4):
                    nc.tensor.matmul(saps,
                                     xcm_t[p][:, cc, si, 128 * pc:128 * (pc + 1)],
                                     wvt[:, cc, :], start=(cc == 0), stop=(cc == 3))
                nc.scalar.activation(e_all[:, si, pc, :], saps, AF.Exp,
                                     bias=zeros, scale=s_all[:, s, pc:pc + 1])

        # --- bilinear upsample of mm_q (+ba) ---
        upB = ps.tile([128, 2, 8], fp, tag="sm", bufs=3, name=f"upB_{p}")
        for si in range(2):
            for pc in range(8):
                for qc in range(2):
                    nc.tensor.matmul(upB[:, si, pc:pc + 1],
                                     bup[:, qc, 128 * pc:128 * (pc + 1)],
                                     mmq_sb[p][:, 2 * si + qc:2 * si + qc + 1],
                                     start=(qc == 0), stop=(qc == 1),
                                     skip_group_check=True)
        mmup = small.tile([128, 2, 8], fp, tag="mmup")
        nc.vector.tensor_scalar(mmup, upB, ba_bc, None, op0=OP.add)

        # --- softmax * CRN weighting -> a (norms precomputed on host) ---
        av = work.tile([128, 2, 8, K], bf, tag="a_s")
        se = small.tile([128, 2, 8], fp, tag="se")
        for si in range(2):
            nc.vector.tensor_reduce(se[:, si], e_all[:, si],
                                    axis=mybir.AxisListType.X, op=OP.add)
        rse = small.tile([128, 2, 8], fp, tag="rse")
        nc.vector.reciprocal(rse, se)
        gcol2 = small.tile([128, 2, 8], fp, tag="gcol2")
        nc.vector.tensor_tensor(gcol2, mmup,
                                s_all[:, 2 * p:2 * p + 2, :], op=OP.mult)
        for si in range(2):
            for pc in range(8):
                nc.vector.tensor_scalar(av[:, si, pc, :], e_all[:, si, pc, :],
                                        rse[:, si, pc:pc + 1],
                                        gcol2[:, si, pc:pc + 1],
                                        op0=OP.mult, op1=OP.mult)
        a_s[p] = av

    def emit_tailB(p):
        """VLAD GEMMs for pair p + centroid subtraction + Square+accum."""
        vlps = ps.tile([128, C], fp, tag="vlps", bufs=1, name=f"vlps_{p}")
        asps = ps.tile([128, 1], fp, tag="sm", bufs=3, name=f"asps_{p}")
        for pc in range(8):
            for si in range(2):
                s = 2 * p + si
                nc.tensor.matmul(vlps[64 * si:64 * si + 64, :],
                                 a_s[p][:, si, pc, :], xpm[:, s, pc, 0:C],
                                 start=(pc == 0), stop=(pc == 7),
                                 skip_group_check=True,
                                 tile_position=(0, 64 * si))
                nc.tensor.matmul(asps[64 * si:64 * si + 64, :],
                                 a_s[p][:, si, pc, :], xpm[:, s, pc, C:C + 1],
                                 start=(pc == 0), stop=(pc == 7),
                                 skip_group_check=True,
                                 tile_position=(0, 64 * si))
        asum = small.tile([128, 1], fp, tag="asum")
        nc.vector.tensor_copy(asum, asps)
        scr128 = work.tile([128, C], fp, tag="scr128")
        nc.vector.tensor_scalar(scr128, cent2, asum, None, op0=OP.mult)
        nc.vector.tensor_tensor(vlad_all[:, p, :], vlps, scr128, op=OP.subtract)
        sq2 = work.tile([128, C], bf, tag="sq2", bufs=2)
        nc.scalar.activation(sq2, vlad_all[:, p, :], AF.Square,
                             bias=zeros, accum_out=q2_all[:, p:p + 1])
        # per-pair intra-norm partials (hides the epilogue chain)
        nc.scalar.activation(nkk_all[:, p:p + 1], q2_all[:, p:p + 1], AF.Sqrt,
                             bias=zeros)
        nc.vector.tensor_scalar(nkk_all[:, p:p + 1], nkk_all[:, p:p + 1], EPS,
                                None, op0=OP.max)
        nc.vector.reciprocal(rrk_all[:, p:p + 1], nkk_all[:, p:p + 1])
        nc.vector.tensor_tensor(ttk_all[:, p:p + 1], q2_all[:, p:p + 1],
                                rrk_all[:, p:p + 1], op=OP.mult)
        nc.vector.tensor_tensor(ttk_all[:, p:p + 1], ttk_all[:, p:p + 1],
                                rrk_all[:, p:p + 1], op=OP.mult)

    # =========================  pipeline schedule  =========================
    emit_loads(0)   # xcm(0) already in flight; this adds pair 0's xpm
    emit_loads(1)
    emit_pool(0)
    for p in range(NPAIR):
        emit_loads(p + 2)
        banks = emit_conv(p)
        emit_pool(p + 1)
        emit_combine(p, banks)
        if p >= 1:
            emit_tailA(p - 1)
        if p >= 2:
            emit_tailB(p - 2)
    emit_tailA(NPAIR - 1)
    emit_tailB(NPAIR - 2)
    emit_tailB(NPAIR - 1)

    # =====================  batched normalization tail  =====================
    gnps = ps.tile([2, NPAIR], fp, tag="sm", bufs=3)
    nc.tensor.matmul(gnps, mask2, ttk_all, start=True, stop=True)
    gs = small.tile([2, NPAIR], fp, tag="gs")
    nc.scalar.activation(gs, gnps, AF.Sqrt, bias=zeros[0:2, :])
    nc.vector.tensor_scalar(gs, gs, EPS, None, op0=OP.max)
    gr = small.tile([2, NPAIR], fp, tag="gr")
    nc.vector.reciprocal(gr, gs)
    gbps = ps.tile([128, NPAIR], fp, tag="sm", bufs=3)
    nc.tensor.matmul(gbps, mask2T, gr, start=True, stop=True)
    rfin = small.tile([128, NPAIR], fp, tag="rfin")
    nc.vector.tensor_tensor(rfin, rrk_all, gbps, op=OP.mult)
    for p in range(NPAIR):
        outf = work.tile([128, C], fp, tag="outf")
        nc.vector.tensor_scalar(outf, vlad_all[:, p, :], rfin[:, p:p + 1], None,
                                op0=OP.mult)
        nc.sync.dma_start(
            out=d["out"][2 * p:2 * p + 2].rearrange("s k c -> (s k) c"),
            in_=outf)

    ctx.close()


# --------------------------------------------------------------------------
# host side
# --------------------------------------------------------------------------

def prep_params(w1, b1, w2, b2, w3, b3, wa, ba, wv, centroids):
    """Build the replicated device parameter tensors (numpy, host-side)."""
    w1q = (w1 * 0.25).astype(F32)
    w2q = (w2 * 0.25).astype(F32)
    w3q = (w3 * 0.25).astype(F32)
    wtv2 = np.zeros((128, 4, NSLOT, 32), BF16)
    for sl, (conv, ty, tx, bank, st) in enumerate(SLOTS):
        m = np.zeros((C, 32), F32)
        if conv == 3:
            m[:, 0:20] = w3q[:, :, ty + 3, tx + 3].T
        elif conv == 2:
            m[:, 0:32] = w2q[:, :, ty + 2, tx + 2].T
        else:
            m[:, 0:32] = w1q[:, :, ty + 1, tx + 1].T
        wtv2[:, :, sl, :] = m.reshape(4, 128, 32).transpose(1, 0, 2).astype(BF16)
    wvt = wv.T.reshape(4, 128, K).transpose(1, 0, 2).astype(BF16)
    # relu96 row layout: [o2(0:32) | o1(32:64) | o3(64:84) | pad(84:96)]
    z12 = np.zeros(12, F32)
    bias96 = np.concatenate([b2, b1, b3, z12]).astype(F32)[:, None]
    war = np.concatenate([wa[0, 32:64, 0, 0], wa[0, 0:32, 0, 0],
                          wa[0, 64:84, 0, 0], z12]).astype(BF16)[:, None]
    R = upsample_matrix_16_to_32()
    B = np.kron(R, R)                                       # (1024, 256)
    bup = B.T.reshape(2, 128, P).transpose(1, 0, 2).astype(BF16)
    cent2 = np.concatenate([centroids, centroids], axis=0).astype(F32)
    mask2 = np.zeros((128, 2), F32)
    mask2[0:64, 0] = 1.0
    mask2[64:128, 1] = 1.0
    return {
        "wtv2": wtv2,
        "wvt": np.ascontiguousarray(wvt),
        "bias96": bias96,
        "war": war,
        "bup": np.ascontiguousarray(bup),
        "cent2": cent2,
        "ba": ba.astype(F32).reshape(1, 1),
        "mask2": mask2,
        "mask2T": np.ascontiguousarray(mask2.T),
    }


_NC_CACHE = None


def _get_nc():
    global _NC_CACHE
    if _NC_CACHE is None:
        _NC_CACHE = build_nc()
    return _NC_CACHE


def make_in_maps(x, params):
    x_r = x.reshape(N, C, P)
    x_bf = x_r.astype(BF16)
    # host 2x2 sum-pool (0.25 is folded into conv weights), zero-padded to 22x22
    x4 = x.reshape(N, C, 16, 2, 16, 2)
    m = x4.sum(axis=(3, 5), dtype=F32)
    mpad = np.zeros((N, C, 22, 22), BF16)
    mpad[:, :, 3:19, 3:19] = m
    mpad = mpad.reshape(N, C, 484)
    nrm = np.sqrt(np.einsum("ncp,ncp->np", x_r, x_r, dtype=np.float64))
    sall = (1.0 / np.maximum(nrm, EPS)).astype(F32)
    xpm_full = np.empty((N, P, C + 1), BF16)
    xpm_full[:, :, 0:C] = x_bf.transpose(0, 2, 1)
    xpm_full[:, :, C] = nrm.astype(BF16)
    in_maps = []
    for core in range(NCORES):
        sl = slice(core * SPC, (core + 1) * SPC)
        in_maps.append({
            "xcm": np.ascontiguousarray(x_bf[sl]),
            "xpm": np.ascontiguousarray(xpm_full[sl]),
            "sall": np.ascontiguousarray(sall[sl]),
            "mpad": np.ascontiguousarray(mpad[sl]),
            **params,
        })
    return in_maps


def kernel(x, w1, b1, w2, b2, w3, b3, wa, ba, wv, centroids, **_ignored):
    x = np.asarray(x, F32)
    params = prep_params(
        np.asarray(w1, F32), np.asarray(b1, F32), np.asarray(w2, F32),
        np.asarray(b2, F32), np.asarray(w3, F32), np.asarray(b3, F32),
        np.asarray(wa, F32), np.asarray(ba, F32), np.asarray(wv, F32),
        np.asarray(centroids, F32))
    nc = _get_nc()
    res = run_bass_kernel_spmd(nc, make_in_maps(x, params), list(range(NCORES)))
    out = np.concatenate([r["out"].reshape(SPC, K * C) for r in res.results], axis=0)
    return out.astype(F32)
